# revision 21
# baseline (speedup 1.0000x reference)
"""Trainium2 Bass kernel for nn_Polarization (segment_reduce).

Reference computation:
    qc  = q - mean(q)
    pol = segment_sum(qc[:, None] * positions, batch, num_segments=1024)

Strategy
--------
`batch` is sorted, so segment sums are differences of prefix sums at the 1024
segment boundaries.  The device streams `positions` and `q` once and produces
fine-grained *group sums* (sums of q*r and of r over each run of L=32
consecutive nodes).  The segment ids themselves never need to be read on
device: the host finds the boundary of every segment with a binary search
(np.searchsorted over the sorted `batch`), then assembles the 1024 segment
sums from the group sums (float64) plus an exact partial-group correction at
each boundary (touches <= 1023*L nodes).  The global mean enters through the
identity  sum_g (q - mu) * r  =  sum_g q*r - mu * sum_g r,  so the device
computes both q*r and r group sums in one pass.

Device kernel (SPMD, identical program on 8 cores, each core gets 1/8 of the
nodes):
  - DMA positions/q for a "big tile" (P=128 partitions x W*L nodes each,
    fully contiguous per partition -> line-rate DMA).
  - VectorE: products q*r (one tensor_tensor mult per big tile).
  - TensorE: grouped reduction over l in [0, L): L accumulating matmuls with
    an identity stationary operand reduce products and positions into PSUM
    [128, W*2*3] (the matmul with lhsT=I is a PSUM-accumulating copy).
  - ScalarE: PSUM -> SBUF drain, then DMA out the tiny group-sum tensor.

Everything heavy (reading 128 MiB, products, reductions) happens on device;
the host does O(segments + boundaries*L) work in numpy.
"""

import os

import numpy as np

P = 128          # SBUF partitions
W = 64           # groups per partition per big tile
L = 32           # nodes per group (device reduction granularity)
NT = 4           # big tiles per core
GP_W = 56        # w-columns whose products run on GPSIMD (rest on DVE)
NUM_CORES = 8
NUM_GRAPHS = 1024
BT_NODES = P * W * L              # nodes per big tile (262144)
NC_NODES = NT * BT_NODES          # nodes per core (1048576)
N_NODES = NUM_CORES * NC_NODES    # 8388608

_NC_CACHE = {}
LAST_RESULTS = None


def _build_nc(nt=NT, w=W, l=L, gp_w=GP_W):
    import concourse.bacc as bacc
    import concourse.mybir as mybir
    import concourse.tile as tile
    from concourse.masks import make_identity

    f32 = mybir.dt.float32
    gp_w = min(gp_w, (2 * w) // 3)
    nc_nodes = nt * P * w * l

    nc = bacc.Bacc(
        "TRN2", target_bir_lowering=False, debug=False, num_devices=NUM_CORES
    )
    pos = nc.dram_tensor("positions", [nc_nodes * 3], f32, kind="ExternalInput")
    q = nc.dram_tensor("q", [nc_nodes], f32, kind="ExternalInput")
    out_qr = nc.dram_tensor("out_qr", [P, nt, w, 3], f32, kind="ExternalOutput")
    out_r = nc.dram_tensor("out_r", [P, nt, w, 3], f32, kind="ExternalOutput")

    # node(t, p, wi, li) = t*BT + p*(w*l) + wi*l + li  -> per-partition data is
    # fully contiguous in DRAM, so the DMA runs at line rate.
    pos_v = pos[:].rearrange("(t p w l c) -> t p w l c", t=nt, p=P, w=w, l=l, c=3)
    q_v = q[:].rearrange("(t p w l) -> t p w l", t=nt, p=P, w=w, l=l)

    bf16 = mybir.dt.bfloat16
    with tile.TileContext(nc) as tc:
        with (
            tc.tile_pool(name="const", bufs=1) as constp,
            tc.tile_pool(name="data", bufs=3) as data,
            tc.tile_pool(name="psum", bufs=2, space="PSUM") as psump,
        ):
            # bf16 identity: fp32 matmuls stream at 1/4 rate, bf16 at full
            # rate.  bf16 only carries the r-sums (they feed the tiny
            # mu * sum(r) correction, where 2^-9 rounding is negligible);
            # the precision-critical q*r sums are reduced on DVE in fp32.
            ident = constp.tile([P, P], bf16)
            make_identity(nc, ident[:])
            # all tiles' group sums staged in SBUF, one batched DMA out
            st_qr = constp.tile([P, nt, w, 3], f32)
            st_r = constp.tile([P, nt, w, 3], f32)
            # lane split: products for w < gp_w on GPSIMD, rest on DVE;
            # each lane has its own DMA chunk so compute starts as soon as
            # its half of the tile has landed.  Bulk position loads ride the
            # SP HWDGE ring; q and result loads ride the ACT ring.
            for t in range(nt):
                pos_f32 = data.tile([P, w, l, 3], f32)
                prod = data.tile([P, w, l, 3], f32)
                qt = data.tile([P, w, l], f32)
                nc.scalar.dma_start(qt[:], q_v[t])
                nc.sync.dma_start(pos_f32[:, :gp_w], pos_v[t][:, :gp_w])
                nc.sync.dma_start(pos_f32[:, gp_w:], pos_v[t][:, gp_w:])
                qb = qt[:].unsqueeze(3).to_broadcast([P, w, l, 3])
                # GPSIMD lane
                nc.gpsimd.tensor_mul(
                    prod[:, :gp_w], pos_f32[:, :gp_w], qb[:, :gp_w]
                )
                # DVE lane
                nc.vector.tensor_mul(
                    prod[:, gp_w:], pos_f32[:, gp_w:], qb[:, gp_w:]
                )
                # DVE: fp32 grouped reduction of q*r over l
                nc.vector.reduce_sum(
                    st_qr[:, t],
                    prod[:].transpose([0, 1, 3, 2]),
                    axis=mybir.AxisListType.X,
                )
                # TensorE: r-sums; lhsT = identity makes each matmul a
                # PSUM-accumulating copy.  The rhs is the truncated-bf16 view
                # of the fp32 positions: the high 2 bytes of an fp32 ARE its
                # round-toward-zero bf16, so a stride-2 bitcast AP avoids any
                # cast pass entirely.
                ps = psump.tile([P, w, 3], f32)
                pos_hi = pos_f32[:].bitcast(bf16)  # [P, w, l, 6]
                for li in range(l):
                    nc.tensor.matmul(
                        ps[:],
                        ident[:],
                        pos_hi[:, :, li, 1::2],
                        start=(li == 0),
                        stop=(li == l - 1),
                    )
                nc.scalar.copy(st_r[:, t], ps[:])
            nc.scalar.dma_start(out_qr[:], st_qr[:])
            nc.scalar.dma_start(out_r[:], st_r[:])
    nc.compile()
    return nc


def _get_nc():
    key = (NT, W, L, GP_W)
    if key not in _NC_CACHE:
        _NC_CACHE[key] = _build_nc(*key)
    return _NC_CACHE[key]


def kernel(positions: np.ndarray, q: np.ndarray, batch: np.ndarray) -> np.ndarray:
    global LAST_RESULTS
    from concourse.bass_utils import run_bass_kernel_spmd

    positions = np.asarray(positions)
    q = np.asarray(q)
    batch = np.asarray(batch)
    assert positions.shape == (N_NODES, 3) and positions.dtype == np.float32
    assert q.shape == (N_NODES,) and q.dtype == np.float32

    # Host: global mean (float64) and segment boundaries via binary search on
    # the sorted segment ids.
    mu = float(q.astype(np.float64).mean())
    bounds = np.searchsorted(batch, np.arange(NUM_GRAPHS + 1)).astype(np.int64)

    nc = _get_nc()
    in_maps = []
    for c in range(NUM_CORES):
        s = c * NC_NODES
        e = s + NC_NODES
        in_maps.append(
            {
                "positions": np.ascontiguousarray(positions[s:e]).reshape(-1),
                "q": np.ascontiguousarray(q[s:e]),
            }
        )
    res = run_bass_kernel_spmd(
        nc,
        in_maps,
        list(range(NUM_CORES)),
        trace=bool(os.environ.get("POL_TRACE")),
    )
    LAST_RESULTS = res

    # Group sums in linear node order: group gi covers nodes [gi*L, gi*L + L),
    # and the device output's natural (t, p, w) order IS linear node order.
    ngroups = N_NODES // L
    ng_core = NC_NODES // L
    Sqr = np.empty((ngroups, 3), np.float64)
    Sr = np.empty((ngroups, 3), np.float64)
    for c in range(NUM_CORES):
        oq = res.results[c]["out_qr"]  # [P, NT, W, 3]
        orr = res.results[c]["out_r"]  # [P, NT, W, 3]
        sl = slice(c * ng_core, (c + 1) * ng_core)
        Sqr[sl] = (
            np.transpose(oq, (1, 0, 2, 3)).astype(np.float64).reshape(ng_core, 3)
        )
        Sr[sl] = (
            np.transpose(orr, (1, 0, 2, 3)).astype(np.float64).reshape(ng_core, 3)
        )

    Cq = np.zeros((ngroups + 1, 3), np.float64)
    Cr = np.zeros((ngroups + 1, 3), np.float64)
    np.cumsum(Sqr, axis=0, out=Cq[1:])
    np.cumsum(Sr, axis=0, out=Cr[1:])

    # Exact partial-group sums at each boundary (<= L-1 nodes each).
    gi = bounds // L
    rem = bounds % L
    idx = np.minimum(gi[:, None] * L + np.arange(L)[None, :], N_NODES - 1)
    mask = (np.arange(L)[None, :] < rem[:, None]).astype(np.float64)
    qs = q[idx].astype(np.float64) * mask            # [1025, L]
    ps_ = positions[idx].astype(np.float64)          # [1025, L, 3]
    part_qr = np.einsum("bg,bgc->bc", qs, ps_)
    part_r = np.einsum("bg,bgc->bc", mask, ps_)

    pre_qr = Cq[gi] + part_qr                        # prefix sums of q*r
    pre_r = Cr[gi] + part_r                          # prefix sums of r
    pol = (pre_qr[1:] - pre_qr[:-1]) - mu * (pre_r[1:] - pre_r[:-1])
    return pol.astype(np.float32)


# revision 22
# speedup vs baseline: 1.2047x; 1.2047x over previous
"""Trainium2 Bass kernel for nn_Polarization (segment_reduce).

Reference computation:
    qc  = q - mean(q)
    pol = segment_sum(qc[:, None] * positions, batch, num_segments=1024)

Strategy
--------
`batch` is sorted, so segment sums are differences of prefix sums at the 1024
segment boundaries.  The device streams `positions` and `q` once and produces
fine-grained *group sums* (sums of q*r and of r over each run of L=32
consecutive nodes).  The segment ids themselves never need to be read on
device: the host finds the boundary of every segment with a binary search
(np.searchsorted over the sorted `batch`), then assembles the 1024 segment
sums from the group sums (float64) plus an exact partial-group correction at
each boundary (touches <= 1023*L nodes).  The global mean enters through the
identity  sum_g (q - mu) * r  =  sum_g q*r - mu * sum_g r,  so the device
computes both q*r and r group sums in one pass.

Device kernel (SPMD, identical program on 8 cores, each core gets 1/8 of the
nodes):
  - DMA positions/q for a "big tile" (P=128 partitions x W*L nodes each,
    fully contiguous per partition -> line-rate DMA).
  - VectorE: products q*r (one tensor_tensor mult per big tile).
  - TensorE: grouped reduction over l in [0, L): L accumulating matmuls with
    an identity stationary operand reduce products and positions into PSUM
    [128, W*2*3] (the matmul with lhsT=I is a PSUM-accumulating copy).
  - ScalarE: PSUM -> SBUF drain, then DMA out the tiny group-sum tensor.

Everything heavy (reading 128 MiB, products, reductions) happens on device;
the host does O(segments + boundaries*L) work in numpy.
"""

import os

import numpy as np

P = 128          # SBUF partitions
W = 64           # groups per partition per big tile
L = 32           # nodes per group (device reduction granularity)
NT = 4           # big tiles per core
GP_W = 56        # w-columns whose products run on GPSIMD (rest on DVE)
NUM_CORES = 8
NUM_GRAPHS = 1024
BT_NODES = P * W * L              # nodes per big tile (262144)
NC_NODES = NT * BT_NODES          # nodes per core (1048576)
N_NODES = NUM_CORES * NC_NODES    # 8388608

_NC_CACHE = {}
LAST_RESULTS = None


def _build_nc(nt=NT, w=W, l=L, gp_w=GP_W):
    import concourse.bacc as bacc
    import concourse.mybir as mybir
    import concourse.tile as tile
    from concourse.masks import make_identity

    f32 = mybir.dt.float32
    gp_w = min(gp_w, (2 * w) // 3)
    nc_nodes = nt * P * w * l

    nc = bacc.Bacc(
        "TRN2", target_bir_lowering=False, debug=False, num_devices=NUM_CORES
    )
    pos = nc.dram_tensor("positions", [nc_nodes * 3], f32, kind="ExternalInput")
    q = nc.dram_tensor("q", [nc_nodes], f32, kind="ExternalInput")
    out_qr = nc.dram_tensor("out_qr", [P, nt, w, 3], f32, kind="ExternalOutput")
    out_r = nc.dram_tensor("out_r", [P, nt, w, 3], f32, kind="ExternalOutput")

    # node(t, p, wi, li) = t*BT + p*(w*l) + wi*l + li  -> per-partition data is
    # fully contiguous in DRAM, so the DMA runs at line rate.
    pos_v = pos[:].rearrange("(t p w l c) -> t p w l c", t=nt, p=P, w=w, l=l, c=3)
    q_v = q[:].rearrange("(t p w l) -> t p w l", t=nt, p=P, w=w, l=l)

    bf16 = mybir.dt.bfloat16
    with tile.TileContext(nc) as tc:
        with (
            tc.tile_pool(name="const", bufs=1) as constp,
            tc.tile_pool(name="data", bufs=3) as data,
            tc.tile_pool(name="psum", bufs=2, space="PSUM") as psump,
        ):
            # bf16 identity: fp32 matmuls stream at 1/4 rate, bf16 at full
            # rate.  bf16 only carries the r-sums (they feed the tiny
            # mu * sum(r) correction, where 2^-9 rounding is negligible);
            # the precision-critical q*r sums are reduced on DVE in fp32.
            ident = constp.tile([P, P], bf16)
            make_identity(nc, ident[:])
            # all tiles' group sums staged in SBUF, one batched DMA out
            st_qr = constp.tile([P, nt, w, 3], f32)
            st_r = constp.tile([P, nt, w, 3], f32)
            # lane split: products for w < gp_w on GPSIMD, rest on DVE;
            # each lane has its own DMA chunk so compute starts as soon as
            # its half of the tile has landed.  Bulk position loads ride the
            # SP HWDGE ring; q and result loads ride the ACT ring.
            for t in range(nt):
                pos_f32 = data.tile([P, w, l, 3], f32)
                prod = data.tile([P, w, l, 3], f32)
                qt = data.tile([P, w, l], f32)
                nc.scalar.dma_start(qt[:], q_v[t])
                nc.sync.dma_start(pos_f32[:, :gp_w], pos_v[t][:, :gp_w])
                nc.sync.dma_start(pos_f32[:, gp_w:], pos_v[t][:, gp_w:])
                qb = qt[:].unsqueeze(3).to_broadcast([P, w, l, 3])
                # GPSIMD lane
                nc.gpsimd.tensor_mul(
                    prod[:, :gp_w], pos_f32[:, :gp_w], qb[:, :gp_w]
                )
                # DVE lane
                nc.vector.tensor_mul(
                    prod[:, gp_w:], pos_f32[:, gp_w:], qb[:, gp_w:]
                )
                # DVE: fp32 grouped reductions of q*r over l (per lane, so the
                # DVE half reduces while GPSIMD still works on its half)
                nc.vector.reduce_sum(
                    st_qr[:, t, gp_w:],
                    prod[:, gp_w:].transpose([0, 1, 3, 2]),
                    axis=mybir.AxisListType.X,
                )
                nc.vector.reduce_sum(
                    st_qr[:, t, :gp_w],
                    prod[:, :gp_w].transpose([0, 1, 3, 2]),
                    axis=mybir.AxisListType.X,
                )
                # TensorE: r-sums; lhsT = identity makes each matmul a
                # PSUM-accumulating copy.  The rhs is the truncated-bf16 view
                # of the fp32 positions: the high 2 bytes of an fp32 ARE its
                # round-toward-zero bf16, so a stride-2 bitcast AP avoids any
                # cast pass entirely.
                ps = psump.tile([P, w, 3], f32)
                pos_hi = pos_f32[:].bitcast(bf16)  # [P, w, l, 6]
                for li in range(l):
                    nc.tensor.matmul(
                        ps[:],
                        ident[:],
                        pos_hi[:, :, li, 1::2],
                        start=(li == 0),
                        stop=(li == l - 1),
                    )
                nc.scalar.copy(st_r[:, t], ps[:])
            nc.scalar.dma_start(out_qr[:], st_qr[:])
            nc.scalar.dma_start(out_r[:], st_r[:])
    nc.compile()
    return nc


def _get_nc():
    key = (NT, W, L, GP_W)
    if key not in _NC_CACHE:
        _NC_CACHE[key] = _build_nc(*key)
    return _NC_CACHE[key]


def kernel(positions: np.ndarray, q: np.ndarray, batch: np.ndarray) -> np.ndarray:
    global LAST_RESULTS
    from concourse.bass_utils import run_bass_kernel_spmd

    positions = np.asarray(positions)
    q = np.asarray(q)
    batch = np.asarray(batch)
    assert positions.shape == (N_NODES, 3) and positions.dtype == np.float32
    assert q.shape == (N_NODES,) and q.dtype == np.float32

    # Host: global mean (float64) and segment boundaries via binary search on
    # the sorted segment ids.
    mu = float(q.astype(np.float64).mean())
    bounds = np.searchsorted(batch, np.arange(NUM_GRAPHS + 1)).astype(np.int64)

    nc = _get_nc()
    in_maps = []
    for c in range(NUM_CORES):
        s = c * NC_NODES
        e = s + NC_NODES
        in_maps.append(
            {
                "positions": np.ascontiguousarray(positions[s:e]).reshape(-1),
                "q": np.ascontiguousarray(q[s:e]),
            }
        )
    res = run_bass_kernel_spmd(
        nc,
        in_maps,
        list(range(NUM_CORES)),
        trace=bool(os.environ.get("POL_TRACE")),
    )
    LAST_RESULTS = res

    # Group sums in linear node order: group gi covers nodes [gi*L, gi*L + L),
    # and the device output's natural (t, p, w) order IS linear node order.
    ngroups = N_NODES // L
    ng_core = NC_NODES // L
    Sqr = np.empty((ngroups, 3), np.float64)
    Sr = np.empty((ngroups, 3), np.float64)
    for c in range(NUM_CORES):
        oq = res.results[c]["out_qr"]  # [P, NT, W, 3]
        orr = res.results[c]["out_r"]  # [P, NT, W, 3]
        sl = slice(c * ng_core, (c + 1) * ng_core)
        Sqr[sl] = (
            np.transpose(oq, (1, 0, 2, 3)).astype(np.float64).reshape(ng_core, 3)
        )
        Sr[sl] = (
            np.transpose(orr, (1, 0, 2, 3)).astype(np.float64).reshape(ng_core, 3)
        )

    Cq = np.zeros((ngroups + 1, 3), np.float64)
    Cr = np.zeros((ngroups + 1, 3), np.float64)
    np.cumsum(Sqr, axis=0, out=Cq[1:])
    np.cumsum(Sr, axis=0, out=Cr[1:])

    # Exact partial-group sums at each boundary (<= L-1 nodes each).
    gi = bounds // L
    rem = bounds % L
    idx = np.minimum(gi[:, None] * L + np.arange(L)[None, :], N_NODES - 1)
    mask = (np.arange(L)[None, :] < rem[:, None]).astype(np.float64)
    qs = q[idx].astype(np.float64) * mask            # [1025, L]
    ps_ = positions[idx].astype(np.float64)          # [1025, L, 3]
    part_qr = np.einsum("bg,bgc->bc", qs, ps_)
    part_r = np.einsum("bg,bgc->bc", mask, ps_)

    pre_qr = Cq[gi] + part_qr                        # prefix sums of q*r
    pre_r = Cr[gi] + part_r                          # prefix sums of r
    pol = (pre_qr[1:] - pre_qr[:-1]) - mu * (pre_r[1:] - pre_r[:-1])
    return pol.astype(np.float32)
